# revision 15
# baseline (speedup 1.0000x reference)
"""Prefix self-attention on 8 TRN2 NeuronCores.

Sharding: core c -> batch b=c//2, query half q=c%2 (1024 queries each).
Each core computes K/V over the full sequence of its batch (redundant
across the core pair), attention + out-proj for its query half only.
No collectives; host concatenates the 8 output shards.

Device layout: activations transposed [feature, time].
  QT/KT: [c_out, t] (2 heads per 128-partition tile)
  V:     [s, d] per head, augmented with a ones-column (denominator) and
         pre-multiplied by the visibility mask (masking + softmax
         normalizer both come out of the same attV matmul).
  scores are computed transposed [s, t] so exp's softmax axis is the
  partition dim -> denominator = one extra matmul row, no DVE reductions.
All matmuls in float32r (full-rate fp32 PE mode, moving free dim >= 256).
"""

import numpy as np
import concourse.bass as bass
import concourse.bacc as bacc
import concourse.tile as tile
from concourse import mybir
from concourse import bass_utils
from contextlib import ExitStack

F32 = mybir.dt.float32
F32R = mybir.dt.float32r

B, T, C = 4, 2048, 1024
H, D, P = 16, 64, 64
T0 = T // 2            # queries per core
S = P + T              # 2112 key positions
NSC = 17               # s-chunks: chunk 0 = prefix (64 rows), 1..16 full 128
NCB = C // 128         # 8 feature blocks
TC = 256               # attention t-chunk
NTC = T0 // TC         # 4
# s-chunk groups per ACT exp; prefix chunk (64 rows) alone so psum
# reads cover exactly the written region (race detector / correctness)
GROUPS = [(0, 1), (1, 5), (5, 9), (9, 13), (13, 17)]

def r(ap):
    return ap.bitcast(F32R)


def build_nc():
    nc = bacc.Bacc("TRN2", target_bir_lowering=False, debug=False)

    xT = nc.dram_tensor("xT", [C, T], F32R, kind="ExternalInput").ap()
    wqT = nc.dram_tensor("wqT", [C, C], F32R, kind="ExternalInput").ap()
    wkT = nc.dram_tensor("wkT", [C, C], F32R, kind="ExternalInput").ap()
    wvT = nc.dram_tensor("wvT", [C, C], F32R, kind="ExternalInput").ap()
    woT = nc.dram_tensor("woT", [C, C], F32R, kind="ExternalInput").ap()
    bqd = nc.dram_tensor("bq", [C], F32, kind="ExternalInput").ap()
    bkd = nc.dram_tensor("bk", [C], F32, kind="ExternalInput").ap()
    bvd = nc.dram_tensor("bv", [C], F32R, kind="ExternalInput").ap()
    bod = nc.dram_tensor("bo", [C], F32, kind="ExternalInput").ap()
    pkT = nc.dram_tensor("pkT", [H, D, P], F32R, kind="ExternalInput").ap()
    pv = nc.dram_tensor("pv", [H, P, D], F32R, kind="ExternalInput").ap()
    visd = nc.dram_tensor("vis", [NSC, 128], F32, kind="ExternalInput").ap()
    onesd = nc.dram_tensor("ones_c", [128, 128], F32R, kind="ExternalInput").ap()
    outT = nc.dram_tensor("outT", [C, T0], F32, kind="ExternalOutput").ap()

    # DRAM views for slab DMAs: [part, cblock, col]
    xT_v = xT.rearrange("(cb p) t -> p cb t", p=128)
    wq_v = wqT.rearrange("(cb p) co -> p cb co", p=128)
    wk_v = wkT.rearrange("(cb p) co -> p cb co", p=128)
    wv_v = wvT.rearrange("(cb p) co -> p cb co", p=128)
    wo_v = woT.rearrange("(cb p) co -> p cb co", p=128)
    outT_v = outT.rearrange("(cb p) t -> p cb t", p=128)

    with tile.TileContext(nc) as tc, ExitStack() as ctx:
        # ---- persistent pools ----
        ktp = ctx.enter_context(tc.tile_pool(name="ktp", bufs=4))
        vp = ctx.enter_context(tc.tile_pool(name="vp", bufs=NSC))
        qtp = ctx.enter_context(tc.tile_pool(name="qtp", bufs=4))
        otp = ctx.enter_context(tc.tile_pool(name="otp", bufs=8))
        smal = ctx.enter_context(tc.tile_pool(name="smal", bufs=1))
        aux = ctx.enter_context(tc.tile_pool(name="aux", bufs=3))
        psA = ctx.enter_context(tc.tile_pool(name="psA", bufs=2, space="PSUM"))
        psB = ctx.enter_context(tc.tile_pool(name="psB", bufs=4, space="PSUM"))

        # ---- constants ----
        vis = smal.tile([128, NSC], F32, name="vis", tag="vis")
        nc.sync.dma_start(vis[:], visd.rearrange("c p -> p c"))
        bkc = smal.tile([128, NCB], F32, name="bkc", tag="bkc")
        nc.sync.dma_start(bkc[:], bkd.rearrange("(m p) -> p m", p=128))
        bqc = smal.tile([128, NCB], F32, name="bqc", tag="bqc")
        nc.sync.dma_start(bqc[:], bqd.rearrange("(m p) -> p m", p=128))
        boc = smal.tile([128, NCB], F32, name="boc", tag="boc")
        nc.sync.dma_start(boc[:], bod.rearrange("(m p) -> p m", p=128))
        bvr = smal.tile([1, C], F32R, name="bvr", tag="bvr")
        nc.sync.dma_start(bvr[:], bvd.rearrange("(a c) -> a c", a=1))
        ones_r = smal.tile([1, 128], F32R, name="ones_r", tag="ones_r")
        nc.sync.dma_start(ones_r[:], onesd[0:1, :])
        ones_b = smal.tile([65, 64], F32R, name="ones_b", tag="ones_b")
        nc.sync.dma_start(ones_b[:], onesd[0:65, 0:64])

        oT = [otp.tile([128, T0], F32R, name=f"oT{k}", tag="ot") for k in range(8)]

        for g in range(2):  # head-group pass: heads 8g..8g+8, c_out g*512..
            co0 = g * 512
            kt = [ktp.tile([128, S], F32R, name=f"kt{g}{m}", tag="kt") for m in range(4)]
            vh = [
                vp.tile([128, 8 * 65], F32R, name=f"vh{g}{c}", tag="vh") for c in range(NSC)
            ]
            qt = [qtp.tile([128, T0], F32R, name=f"qt{g}{m}", tag="qt") for m in range(4)]

            # prefix K/V + ones cols
            for lh in range(8):
                h = 8 * g + lh
                par = lh % 2
                nc.sync.dma_start(
                    kt[lh // 2][par * 64 : par * 64 + 64, 0:P], pkT[h]
                )
                nc.sync.dma_start(vh[0][0:P, lh * 65 : lh * 65 + 64], pv[h])
            for c in range(NSC):
                vv = vh[c].rearrange("p (h x) -> p h x", x=65)
                nc.sync.dma_start(
                    vv[:, :, 64:65],
                    onesd.rearrange("p (a b) -> p a b", b=1)[:, 0:8, :],
                )

            # ---- projections (streamed x and W slabs) ----
            with tc.tile_pool(name="xw", bufs=1) as xwp:
                wks = xwp.tile([128, NCB * 512], F32R, name="wks", tag="w", bufs=3)
                nc.sync.dma_start(
                    wks.rearrange("p (cb co) -> p cb co", co=512),
                    wk_v[:, :, co0 : co0 + 512],
                )
                wvs = xwp.tile([128, NCB * 512], F32R, name="wvs", tag="w", bufs=3)
                nc.sync.dma_start(
                    wvs.rearrange("p (cb co) -> p cb co", co=512),
                    wv_v[:, :, co0 : co0 + 512],
                )
                wqs = xwp.tile([128, NCB * 512], F32R, name="wqs", tag="w", bufs=3)
                nc.sync.dma_start(
                    wqs.rearrange("p (cb co) -> p cb co", co=512),
                    wq_v[:, :, co0 : co0 + 512],
                )
                wks_v = wks.rearrange("p (cb co) -> p cb co", co=512)
                wvs_v = wvs.rearrange("p (cb co) -> p cb co", co=512)
                wqs_v = wqs.rearrange("p (cb co) -> p cb co", co=512)

                for j in range(8):  # t-slabs of 256
                    xsl = xwp.tile([128, NCB * 256], F32R, name="xsl", tag="x", bufs=2)
                    nc.sync.dma_start(
                        xsl.rearrange("p (cb t) -> p cb t", t=256),
                        xT_v[:, :, j * 256 : (j + 1) * 256],
                    )
                    xv = xsl.rearrange("p (cb t) -> p cb t", t=256)

                    # K: out[c_out 128, t 256]
                    for m in range(4):
                        pk = psB.tile([128, 256], F32, name="pk", tag="psB")
                        for cb in range(NCB):
                            nc.tensor.matmul(
                                pk[:],
                                r(wks_v[:, cb, m * 128 : (m + 1) * 128]),
                                r(xv[:, cb, :]),
                                start=(cb == 0),
                                stop=(cb == NCB - 1),
                            )
                        nc.vector.tensor_scalar_add(
                            kt[m][:, P + j * 256 : P + (j + 1) * 256],
                            pk[:],
                            bkc[:, g * 4 + m : g * 4 + m + 1],
                        )

                    # V: out[t 128, c_out 512] (+bias via rank-1)
                    for tb in range(2):
                        pvp = psA.tile([128, 1024], F32, name="pvp", tag="psA")
                        for cb in range(NCB):
                            nc.tensor.matmul(
                                pvp[:, 0:512],
                                r(xv[:, cb, tb * 128 : (tb + 1) * 128]),
                                r(wvs_v[:, cb, :]),
                                start=(cb == 0),
                                stop=False,
                            )
                        nc.tensor.matmul(
                            pvp[:, 0:512],
                            r(ones_r[:, 0:128]),
                            r(bvr[:, co0 : co0 + 512]),
                            start=False,
                            stop=True,
                        )
                        c = 2 * j + tb + 1
                        nc.vector.tensor_copy(
                            vh[c]
                            .rearrange("p (h x) -> p h x", x=65)[:, :, 0:64],
                            pvp[:, 0:512].rearrange("p (h x) -> p h x", x=64),
                        )

                    # Q (first half of t only): out[c_out 128, t 256]
                    if j < 4:
                        for m in range(4):
                            pq = psB.tile([128, 256], F32, name="pq", tag="psB")
                            for cb in range(NCB):
                                nc.tensor.matmul(
                                    pq[:],
                                    r(wqs_v[:, cb, m * 128 : (m + 1) * 128]),
                                    r(xv[:, cb, :]),
                                    start=(cb == 0),
                                    stop=(cb == NCB - 1),
                                )
                            nc.vector.tensor_scalar_add(
                                qt[m][:, j * 256 : (j + 1) * 256],
                                pq[:],
                                bqc[:, g * 4 + m : g * 4 + m + 1],
                            )

            # visibility mask -> V rows (and ones cols; vis^2 == vis).
            # chunk 0: only rows 0:P are ever written/read
            nc.vector.tensor_scalar_mul(vh[0][0:P, :], vh[0][0:P, :], vis[0:P, 0:1])
            for c in range(1, NSC):
                nc.vector.tensor_scalar_mul(vh[c][:], vh[c][:], vis[:, c : c + 1])

            # ---- attention ----
            with tc.tile_pool(name="wtp", bufs=10) as wtp:
                for lh in range(8):
                    gh = 8 * g + lh
                    par = lh % 2
                    ktt = kt[lh // 2]
                    qtt = qt[lh // 2]
                    for tci in range(NTC):
                        t0, t1 = tci * TC, (tci + 1) * TC
                        po = psB.tile([128, 256], F32, name="po", tag="psB")
                        for c0, c1 in GROUPS:
                            nch = c1 - c0
                            pp = P if c0 == 0 else 128  # partitions covered
                            ps = psA.tile([128, 1024], F32, name="ps", tag="psA")
                            for c in range(c0, c1):
                                sl = c - c0
                                if c == 0:
                                    m0, msz = 0, P
                                else:
                                    m0, msz = P + (c - 1) * 128, 128
                                nc.tensor.matmul(
                                    ps[0:msz, sl * TC : sl * TC + TC],
                                    r(ktt[par * 64 : par * 64 + 64, m0 : m0 + msz]),
                                    r(qtt[par * 64 : par * 64 + 64, t0:t1]),
                                    start=True,
                                    stop=True,
                                )
                            wt = wtp.tile([128, 1024], F32R, name="wt", tag="wt")
                            nc.scalar.activation(
                                wt[0:pp, 0 : nch * TC],
                                ps[0:pp, 0 : nch * TC],
                                mybir.ActivationFunctionType.Exp,
                                bias=0.0,
                                scale=0.125,
                            )
                            for c in range(c0, c1):
                                sl = c - c0
                                ksz = P if c == 0 else 128
                                nc.tensor.matmul(
                                    po[0:65, :],
                                    r(vh[c][0:ksz, lh * 65 : lh * 65 + 65]),
                                    r(wt[0:ksz, sl * TC : sl * TC + TC]),
                                    start=(c == 0),
                                    stop=(c == NSC - 1),
                                )
                        # normalize: row 64 of po = denominator
                        rcp = aux.tile([65, 256], F32R, name="rcp", tag="rcp")
                        with nc.allow_low_precision(reason="f32r == f32 bits"):
                            nc.vector.reciprocal(rcp[64:65, :], po[64:65, :])
                        pb = psA.tile([128, 1024], F32, name="pb", tag="psA")
                        nc.tensor.matmul(
                            pb[0:64, 0:256],
                            r(ones_b[64:65, :]),
                            r(rcp[64:65, :]),
                            start=True,
                            stop=True,
                        )
                        rb = aux.tile([64, 256], F32, name="rb", tag="rb")
                        nc.vector.tensor_copy(rb[:], pb[0:64, 0:256])
                        ot = oT[gh // 2]
                        if gh % 2 == 0:
                            nc.vector.tensor_mul(
                                ot[0:64, t0:t1], po[0:64, :], rb[:]
                            )
                        else:
                            tmp = aux.tile([64, 256], F32R, name="tmp", tag="tmp")
                            nc.vector.tensor_mul(tmp[:], po[0:64, :], rb[:])
                            nc.sync.dma_start(ot[64:128, t0:t1], tmp[:])

        # ---- output projection: outT[c_out, t] = WoT.T @ oT (+bo) ----
        with tc.tile_pool(name="wop", bufs=2) as wop:
            for hf in range(2):
                wos = wop.tile([128, NCB * 512], F32R, name="wos", tag="wo")
                nc.sync.dma_start(
                    wos.rearrange("p (cb co) -> p cb co", co=512),
                    wo_v[:, :, hf * 512 : (hf + 1) * 512],
                )
                wos_v = wos.rearrange("p (cb co) -> p cb co", co=512)
                for mo in range(4):
                    cbo = hf * 4 + mo
                    for tj in range(2):
                        pO = psA.tile([128, 1024], F32, name="pO", tag="psA")
                        for cb in range(NCB):
                            nc.tensor.matmul(
                                pO[:, 0:512],
                                r(wos_v[:, cb, mo * 128 : (mo + 1) * 128]),
                                r(oT[cb][:, tj * 512 : (tj + 1) * 512]),
                                start=(cb == 0),
                                stop=(cb == NCB - 1),
                            )
                        st = qtp.tile([128, T0], F32, name="st", tag="qt")
                        nc.vector.tensor_scalar_add(
                            st[:, 0:512], pO[:, 0:512], boc[:, cbo : cbo + 1]
                        )
                        nc.sync.dma_start(
                            outT_v[:, cbo, tj * 512 : (tj + 1) * 512],
                            st[:, 0:512],
                        )
    nc.compile()
    return nc


_NC = None


def _get_nc():
    global _NC
    if _NC is None:
        _NC = build_nc()
    return _NC


def make_in_maps(x, attn_mask, prefix_k, prefix_v, Wq, bq, Wk, bk, Wv, bv, Wo, bo):
    shared = {
        "wqT": np.ascontiguousarray(np.asarray(Wq, np.float32).T),
        "wkT": np.ascontiguousarray(np.asarray(Wk, np.float32).T),
        "wvT": np.ascontiguousarray(np.asarray(Wv, np.float32).T),
        "woT": np.ascontiguousarray(np.asarray(Wo, np.float32).T),
        "bq": np.asarray(bq, np.float32),
        "bk": np.asarray(bk, np.float32),
        "bv": np.asarray(bv, np.float32),
        "bo": np.asarray(bo, np.float32),
    }
    x = np.asarray(x, np.float32)
    attn_mask = np.asarray(attn_mask)
    prefix_k = np.asarray(prefix_k, np.float32)
    prefix_v = np.asarray(prefix_v, np.float32)
    ones_c = np.ones((128, 128), np.float32)
    in_maps = []
    for core in range(8):
        b, half = core // 2, core % 2
        # query half first, other half second (keeps the device program SPMD)
        xp = np.concatenate(
            [x[b, half * T0 : (half + 1) * T0], x[b, (1 - half) * T0 : (2 - half) * T0]]
        )
        m = attn_mask[b, 0, 0].astype(np.float32)
        mperm = np.concatenate(
            [m[half * T0 : (half + 1) * T0], m[(1 - half) * T0 : (2 - half) * T0]]
        )
        vis = np.zeros((NSC, 128), np.float32)
        vis[0, :P] = 1.0
        vis[1:] = mperm.reshape(16, 128)
        in_maps.append(
            dict(
                shared,
                xT=np.ascontiguousarray(xp.T),
                pkT=np.ascontiguousarray(prefix_k[b].transpose(0, 2, 1)),
                pv=np.ascontiguousarray(prefix_v[b]),
                vis=vis,
                ones_c=ones_c,
            )
        )
    return in_maps


def kernel(**inputs) -> np.ndarray:
    nc = _get_nc()
    in_maps = make_in_maps(**inputs)
    res = bass_utils.run_bass_kernel_spmd(nc, in_maps, core_ids=list(range(8)))
    out = np.empty((B, T, C), np.float32)
    for core in range(8):
        b, half = core // 2, core % 2
        out[b, half * T0 : (half + 1) * T0] = res.results[core]["outT"].T
    return out


# revision 16
# speedup vs baseline: 1.0502x; 1.0502x over previous
"""Prefix self-attention on 8 TRN2 NeuronCores.

Sharding: core c -> batch b=c//2, query half q=c%2 (1024 queries each).
Each core computes K/V over the full sequence of its batch (redundant
across the core pair), attention + out-proj for its query half only.
No collectives; host concatenates the 8 output shards.

Device layout: activations transposed [feature, time].
  QT/KT: [c_out, t] (2 heads per 128-partition tile)
  V:     [s, d] per head, augmented with a ones-column (denominator) and
         pre-multiplied by the visibility mask (masking + softmax
         normalizer both come out of the same attV matmul).
  scores are computed transposed [s, t] so exp's softmax axis is the
  partition dim -> denominator = one extra matmul row, no DVE reductions.
All matmuls in float32r (full-rate fp32 PE mode, moving free dim >= 256).
"""

import numpy as np
import concourse.bass as bass
import concourse.bacc as bacc
import concourse.tile as tile
from concourse import mybir
from concourse import bass_utils
from contextlib import ExitStack

F32 = mybir.dt.float32
F32R = mybir.dt.float32r

B, T, C = 4, 2048, 1024
H, D, P = 16, 64, 64
T0 = T // 2            # queries per core
S = P + T              # 2112 key positions
NSC = 17               # s-chunks: chunk 0 = prefix (64 rows), 1..16 full 128
NCB = C // 128         # 8 feature blocks
TC = 256               # attention t-chunk
NTC = T0 // TC         # 4
# s-chunk groups per ACT exp; prefix chunk (64 rows) alone so psum
# reads cover exactly the written region (race detector / correctness)
GROUPS = [(0, 1), (1, 5), (5, 9), (9, 13), (13, 17)]

def r(ap):
    return ap.bitcast(F32R)


def build_nc():
    nc = bacc.Bacc("TRN2", target_bir_lowering=False, debug=False)

    xT = nc.dram_tensor("xT", [C, T], F32R, kind="ExternalInput").ap()
    wqT = nc.dram_tensor("wqT", [C, C], F32R, kind="ExternalInput").ap()
    wkT = nc.dram_tensor("wkT", [C, C], F32R, kind="ExternalInput").ap()
    wvT = nc.dram_tensor("wvT", [C, C], F32R, kind="ExternalInput").ap()
    woT = nc.dram_tensor("woT", [C, C], F32R, kind="ExternalInput").ap()
    bqd = nc.dram_tensor("bq", [C], F32, kind="ExternalInput").ap()
    bkd = nc.dram_tensor("bk", [C], F32, kind="ExternalInput").ap()
    bvd = nc.dram_tensor("bv", [C], F32R, kind="ExternalInput").ap()
    bod = nc.dram_tensor("bo", [C], F32, kind="ExternalInput").ap()
    pkT = nc.dram_tensor("pkT", [H, D, P], F32R, kind="ExternalInput").ap()
    pv = nc.dram_tensor("pv", [H, P, D], F32R, kind="ExternalInput").ap()
    visd = nc.dram_tensor("vis", [NSC, 128], F32, kind="ExternalInput").ap()
    onesd = nc.dram_tensor("ones_c", [128, 128], F32R, kind="ExternalInput").ap()
    outT = nc.dram_tensor("outT", [C, T0], F32, kind="ExternalOutput").ap()

    # DRAM views for slab DMAs: [part, cblock, col]
    xT_v = xT.rearrange("(cb p) t -> p cb t", p=128)
    wq_v = wqT.rearrange("(cb p) co -> p cb co", p=128)
    wk_v = wkT.rearrange("(cb p) co -> p cb co", p=128)
    wv_v = wvT.rearrange("(cb p) co -> p cb co", p=128)
    wo_v = woT.rearrange("(cb p) co -> p cb co", p=128)
    outT_v = outT.rearrange("(cb p) t -> p cb t", p=128)

    with tile.TileContext(nc) as tc, ExitStack() as ctx:
        # ---- persistent pools ----
        ktp = ctx.enter_context(tc.tile_pool(name="ktp", bufs=4))
        vp = ctx.enter_context(tc.tile_pool(name="vp", bufs=NSC))
        qtp = ctx.enter_context(tc.tile_pool(name="qtp", bufs=4))
        otp = ctx.enter_context(tc.tile_pool(name="otp", bufs=8))
        smal = ctx.enter_context(tc.tile_pool(name="smal", bufs=1))
        aux = ctx.enter_context(tc.tile_pool(name="aux", bufs=3))
        psA = ctx.enter_context(tc.tile_pool(name="psA", bufs=2, space="PSUM"))
        psB = ctx.enter_context(tc.tile_pool(name="psB", bufs=4, space="PSUM"))

        # ---- constants ----
        vis = smal.tile([128, NSC], F32, name="vis", tag="vis")
        nc.sync.dma_start(vis[:], visd.rearrange("c p -> p c"))
        bkc = smal.tile([128, NCB], F32, name="bkc", tag="bkc")
        nc.sync.dma_start(bkc[:], bkd.rearrange("(m p) -> p m", p=128))
        bqc = smal.tile([128, NCB], F32, name="bqc", tag="bqc")
        nc.sync.dma_start(bqc[:], bqd.rearrange("(m p) -> p m", p=128))
        boc = smal.tile([128, NCB], F32, name="boc", tag="boc")
        nc.sync.dma_start(boc[:], bod.rearrange("(m p) -> p m", p=128))
        bvr = smal.tile([1, C], F32R, name="bvr", tag="bvr")
        nc.sync.dma_start(bvr[:], bvd.rearrange("(a c) -> a c", a=1))
        ones_r = smal.tile([1, 128], F32R, name="ones_r", tag="ones_r")
        nc.sync.dma_start(ones_r[:], onesd[0:1, :])
        ones_b = smal.tile([65, 64], F32R, name="ones_b", tag="ones_b")
        nc.sync.dma_start(ones_b[:], onesd[0:65, 0:64])

        oT = [otp.tile([128, T0], F32R, name=f"oT{k}", tag="ot") for k in range(8)]

        for g in range(2):  # head-group pass: heads 8g..8g+8, c_out g*512..
            co0 = g * 512
            kt = [ktp.tile([128, S], F32R, name=f"kt{g}{m}", tag="kt") for m in range(4)]
            vh = [
                vp.tile([128, 8 * 65], F32R, name=f"vh{g}{c}", tag="vh") for c in range(NSC)
            ]
            qt = [qtp.tile([128, T0], F32R, name=f"qt{g}{m}", tag="qt") for m in range(4)]

            # prefix K/V + ones cols
            for lh in range(8):
                h = 8 * g + lh
                par = lh % 2
                nc.sync.dma_start(
                    kt[lh // 2][par * 64 : par * 64 + 64, 0:P], pkT[h]
                )
                nc.sync.dma_start(vh[0][0:P, lh * 65 : lh * 65 + 64], pv[h])
            for c in range(NSC):
                vv = vh[c].rearrange("p (h x) -> p h x", x=65)
                nc.sync.dma_start(
                    vv[:, :, 64:65],
                    onesd.rearrange("p (a b) -> p a b", b=1)[:, 0:8, :],
                )

            # ---- projections (streamed x and W slabs) ----
            with tc.tile_pool(name="xw", bufs=1) as xwp:
                wks = xwp.tile([128, NCB * 512], F32R, name="wks", tag="w", bufs=3)
                nc.sync.dma_start(
                    wks.rearrange("p (cb co) -> p cb co", co=512),
                    wk_v[:, :, co0 : co0 + 512],
                )
                wvs = xwp.tile([128, NCB * 512], F32R, name="wvs", tag="w", bufs=3)
                nc.sync.dma_start(
                    wvs.rearrange("p (cb co) -> p cb co", co=512),
                    wv_v[:, :, co0 : co0 + 512],
                )
                wqs = xwp.tile([128, NCB * 512], F32R, name="wqs", tag="w", bufs=3)
                nc.sync.dma_start(
                    wqs.rearrange("p (cb co) -> p cb co", co=512),
                    wq_v[:, :, co0 : co0 + 512],
                )
                wks_v = wks.rearrange("p (cb co) -> p cb co", co=512)
                wvs_v = wvs.rearrange("p (cb co) -> p cb co", co=512)
                wqs_v = wqs.rearrange("p (cb co) -> p cb co", co=512)

                for j in range(8):  # t-slabs of 256
                    xsl = xwp.tile([128, NCB * 256], F32R, name="xsl", tag="x", bufs=2)
                    nc.sync.dma_start(
                        xsl.rearrange("p (cb t) -> p cb t", t=256),
                        xT_v[:, :, j * 256 : (j + 1) * 256],
                    )
                    xv = xsl.rearrange("p (cb t) -> p cb t", t=256)

                    # K: out[c_out 128, t 256]
                    for m in range(4):
                        pk = psB.tile([128, 256], F32, name="pk", tag="psB")
                        for cb in range(NCB):
                            nc.tensor.matmul(
                                pk[:],
                                r(wks_v[:, cb, m * 128 : (m + 1) * 128]),
                                r(xv[:, cb, :]),
                                start=(cb == 0),
                                stop=(cb == NCB - 1),
                            )
                        nc.vector.tensor_scalar_add(
                            kt[m][:, P + j * 256 : P + (j + 1) * 256],
                            pk[:],
                            bkc[:, g * 4 + m : g * 4 + m + 1],
                        )

                    # V: out[t 128, c_out 512] (+bias via rank-1)
                    for tb in range(2):
                        pvp = psA.tile([128, 1024], F32, name="pvp", tag="psA")
                        for cb in range(NCB):
                            nc.tensor.matmul(
                                pvp[:, 0:512],
                                r(xv[:, cb, tb * 128 : (tb + 1) * 128]),
                                r(wvs_v[:, cb, :]),
                                start=(cb == 0),
                                stop=False,
                            )
                        nc.tensor.matmul(
                            pvp[:, 0:512],
                            r(ones_r[:, 0:128]),
                            r(bvr[:, co0 : co0 + 512]),
                            start=False,
                            stop=True,
                        )
                        c = 2 * j + tb + 1
                        nc.vector.tensor_copy(
                            vh[c]
                            .rearrange("p (h x) -> p h x", x=65)[:, :, 0:64],
                            pvp[:, 0:512].rearrange("p (h x) -> p h x", x=64),
                        )

                    # Q (first half of t only): out[c_out 128, t 256]
                    if j < 4:
                        for m in range(4):
                            pq = psB.tile([128, 256], F32, name="pq", tag="psB")
                            for cb in range(NCB):
                                nc.tensor.matmul(
                                    pq[:],
                                    r(wqs_v[:, cb, m * 128 : (m + 1) * 128]),
                                    r(xv[:, cb, :]),
                                    start=(cb == 0),
                                    stop=(cb == NCB - 1),
                                )
                            nc.vector.tensor_scalar_add(
                                qt[m][:, j * 256 : (j + 1) * 256],
                                pq[:],
                                bqc[:, g * 4 + m : g * 4 + m + 1],
                            )

            # visibility mask -> V rows (and ones cols; vis^2 == vis).
            # chunk 0: only rows 0:P are ever written/read
            nc.vector.tensor_scalar_mul(vh[0][0:P, :], vh[0][0:P, :], vis[0:P, 0:1])
            for c in range(1, NSC):
                nc.vector.tensor_scalar_mul(vh[c][:], vh[c][:], vis[:, c : c + 1])

            # ---- attention ----
            # head PAIRS interleaved: even head on PE row-group 0, odd on
            # row-group 1 -> concurrent scores matmuls, denser PE stream
            # (keeps the HAM clock gate open)
            with tc.tile_pool(name="wtp", bufs=10) as wtp:
                for pr in range(4):
                    ktt = kt[pr]
                    qtt = qt[pr]
                    for tci in range(NTC):
                        t0, t1 = tci * TC, (tci + 1) * TC
                        po2 = [
                            psB.tile([128, 256], F32, name=f"po{i}", tag="psB")
                            for i in range(2)
                        ]
                        for c0, c1 in GROUPS:
                            nch = c1 - c0
                            pp = P if c0 == 0 else 128  # partitions covered
                            ps2 = [
                                psA.tile([128, 1024], F32, name=f"ps{i}", tag="psA")
                                for i in range(2)
                            ]
                            for c in range(c0, c1):
                                sl = c - c0
                                if c == 0:
                                    m0, msz = 0, P
                                else:
                                    m0, msz = P + (c - 1) * 128, 128
                                for i in range(2):
                                    nc.tensor.matmul(
                                        ps2[i][0:msz, sl * TC : sl * TC + TC],
                                        r(ktt[i * 64 : i * 64 + 64, m0 : m0 + msz]),
                                        r(qtt[i * 64 : i * 64 + 64, t0:t1]),
                                        start=True,
                                        stop=True,
                                    )
                            wt2 = []
                            for i in range(2):
                                wt = wtp.tile([128, 1024], F32R, name=f"wt{i}", tag="wt")
                                wt2.append(wt)
                                nc.scalar.activation(
                                    wt[0:pp, 0 : nch * TC],
                                    ps2[i][0:pp, 0 : nch * TC],
                                    mybir.ActivationFunctionType.Exp,
                                    bias=0.0,
                                    scale=0.125,
                                )
                            for c in range(c0, c1):
                                sl = c - c0
                                ksz = P if c == 0 else 128
                                for i in range(2):
                                    lh = 2 * pr + i
                                    nc.tensor.matmul(
                                        po2[i][0:65, :],
                                        r(vh[c][0:ksz, lh * 65 : lh * 65 + 65]),
                                        r(wt2[i][0:ksz, sl * TC : sl * TC + TC]),
                                        start=(c == 0),
                                        stop=(c == NSC - 1),
                                    )
                        for i in range(2):
                            po = po2[i]
                            gh = 8 * g + 2 * pr + i
                            # normalize: row 64 of po = denominator
                            rcp = aux.tile([65, 256], F32R, name="rcp", tag="rcp")
                            with nc.allow_low_precision(reason="f32r == f32 bits"):
                                nc.vector.reciprocal(rcp[64:65, :], po[64:65, :])
                            pb = psA.tile([128, 1024], F32, name="pb", tag="psA")
                            nc.tensor.matmul(
                                pb[0:64, 0:256],
                                r(ones_b[64:65, :]),
                                r(rcp[64:65, :]),
                                start=True,
                                stop=True,
                            )
                            rb = aux.tile([64, 256], F32, name="rb", tag="rb")
                            nc.vector.tensor_copy(rb[:], pb[0:64, 0:256])
                            ot = oT[gh // 2]
                            if gh % 2 == 0:
                                nc.vector.tensor_mul(
                                    ot[0:64, t0:t1], po[0:64, :], rb[:]
                                )
                            else:
                                tmp = aux.tile([64, 256], F32R, name="tmp", tag="tmp")
                                nc.vector.tensor_mul(tmp[:], po[0:64, :], rb[:])
                                nc.sync.dma_start(ot[64:128, t0:t1], tmp[:])

        # ---- output projection: outT[c_out, t] = WoT.T @ oT (+bo) ----
        with tc.tile_pool(name="wop", bufs=2) as wop:
            for hf in range(2):
                wos = wop.tile([128, NCB * 512], F32R, name="wos", tag="wo")
                nc.sync.dma_start(
                    wos.rearrange("p (cb co) -> p cb co", co=512),
                    wo_v[:, :, hf * 512 : (hf + 1) * 512],
                )
                wos_v = wos.rearrange("p (cb co) -> p cb co", co=512)
                for mo in range(4):
                    cbo = hf * 4 + mo
                    for tj in range(2):
                        pO = psA.tile([128, 1024], F32, name="pO", tag="psA")
                        for cb in range(NCB):
                            nc.tensor.matmul(
                                pO[:, 0:512],
                                r(wos_v[:, cb, mo * 128 : (mo + 1) * 128]),
                                r(oT[cb][:, tj * 512 : (tj + 1) * 512]),
                                start=(cb == 0),
                                stop=(cb == NCB - 1),
                            )
                        st = qtp.tile([128, T0], F32, name="st", tag="qt")
                        nc.vector.tensor_scalar_add(
                            st[:, 0:512], pO[:, 0:512], boc[:, cbo : cbo + 1]
                        )
                        nc.sync.dma_start(
                            outT_v[:, cbo, tj * 512 : (tj + 1) * 512],
                            st[:, 0:512],
                        )
    nc.compile()
    return nc


_NC = None


def _get_nc():
    global _NC
    if _NC is None:
        _NC = build_nc()
    return _NC


def make_in_maps(x, attn_mask, prefix_k, prefix_v, Wq, bq, Wk, bk, Wv, bv, Wo, bo):
    shared = {
        "wqT": np.ascontiguousarray(np.asarray(Wq, np.float32).T),
        "wkT": np.ascontiguousarray(np.asarray(Wk, np.float32).T),
        "wvT": np.ascontiguousarray(np.asarray(Wv, np.float32).T),
        "woT": np.ascontiguousarray(np.asarray(Wo, np.float32).T),
        "bq": np.asarray(bq, np.float32),
        "bk": np.asarray(bk, np.float32),
        "bv": np.asarray(bv, np.float32),
        "bo": np.asarray(bo, np.float32),
    }
    x = np.asarray(x, np.float32)
    attn_mask = np.asarray(attn_mask)
    prefix_k = np.asarray(prefix_k, np.float32)
    prefix_v = np.asarray(prefix_v, np.float32)
    ones_c = np.ones((128, 128), np.float32)
    in_maps = []
    for core in range(8):
        b, half = core // 2, core % 2
        # query half first, other half second (keeps the device program SPMD)
        xp = np.concatenate(
            [x[b, half * T0 : (half + 1) * T0], x[b, (1 - half) * T0 : (2 - half) * T0]]
        )
        m = attn_mask[b, 0, 0].astype(np.float32)
        mperm = np.concatenate(
            [m[half * T0 : (half + 1) * T0], m[(1 - half) * T0 : (2 - half) * T0]]
        )
        vis = np.zeros((NSC, 128), np.float32)
        vis[0, :P] = 1.0
        vis[1:] = mperm.reshape(16, 128)
        in_maps.append(
            dict(
                shared,
                xT=np.ascontiguousarray(xp.T),
                pkT=np.ascontiguousarray(prefix_k[b].transpose(0, 2, 1)),
                pv=np.ascontiguousarray(prefix_v[b]),
                vis=vis,
                ones_c=ones_c,
            )
        )
    return in_maps


def kernel(**inputs) -> np.ndarray:
    nc = _get_nc()
    in_maps = make_in_maps(**inputs)
    res = bass_utils.run_bass_kernel_spmd(nc, in_maps, core_ids=list(range(8)))
    out = np.empty((B, T, C), np.float32)
    for core in range(8):
        b, half = core // 2, core % 2
        out[b, half * T0 : (half + 1) * T0] = res.results[core]["outT"].T
    return out


# revision 17
# speedup vs baseline: 1.0652x; 1.0143x over previous
"""Prefix self-attention on 8 TRN2 NeuronCores.

Sharding: core c -> batch b=c//2, query half q=c%2 (1024 queries each).
Each core computes K/V over the full sequence of its batch (redundant
across the core pair), attention + out-proj for its query half only.
No collectives; host concatenates the 8 output shards.

Device layout: activations transposed [feature, time].
  QT/KT: [c_out, t] (2 heads per 128-partition tile)
  V:     [s, d] per head, augmented with a ones-column (denominator) and
         pre-multiplied by the visibility mask (masking + softmax
         normalizer both come out of the same attV matmul).
  scores are computed transposed [s, t] so exp's softmax axis is the
  partition dim -> denominator = one extra matmul row, no DVE reductions.
All matmuls in float32r (full-rate fp32 PE mode, moving free dim >= 256).
"""

import numpy as np
import concourse.bass as bass
import concourse.bacc as bacc
import concourse.tile as tile
from concourse import mybir
from concourse import bass_utils
from contextlib import ExitStack

F32 = mybir.dt.float32
F32R = mybir.dt.float32r

B, T, C = 4, 2048, 1024
H, D, P = 16, 64, 64
T0 = T // 2            # queries per core
S = P + T              # 2112 key positions
NSC = 17               # s-chunks: chunk 0 = prefix (64 rows), 1..16 full 128
NCB = C // 128         # 8 feature blocks
TC = 256               # attention t-chunk
NTC = T0 // TC         # 4
# s-chunk groups per ACT exp; prefix chunk (64 rows) alone so psum
# reads cover exactly the written region (race detector / correctness)
GROUPS = [(0, 1)] + [(c, c + 2) for c in range(1, 17, 2)]

def r(ap):
    return ap.bitcast(F32R)


def build_nc():
    nc = bacc.Bacc("TRN2", target_bir_lowering=False, debug=False)

    xT = nc.dram_tensor("xT", [C, T], F32R, kind="ExternalInput").ap()
    wqT = nc.dram_tensor("wqT", [C, C], F32R, kind="ExternalInput").ap()
    wkT = nc.dram_tensor("wkT", [C, C], F32R, kind="ExternalInput").ap()
    wvT = nc.dram_tensor("wvT", [C, C], F32R, kind="ExternalInput").ap()
    woT = nc.dram_tensor("woT", [C, C], F32R, kind="ExternalInput").ap()
    bqd = nc.dram_tensor("bq", [C], F32, kind="ExternalInput").ap()
    bkd = nc.dram_tensor("bk", [C], F32, kind="ExternalInput").ap()
    bvd = nc.dram_tensor("bv", [C], F32R, kind="ExternalInput").ap()
    bod = nc.dram_tensor("bo", [C], F32, kind="ExternalInput").ap()
    pkT = nc.dram_tensor("pkT", [H, D, P], F32R, kind="ExternalInput").ap()
    pv = nc.dram_tensor("pv", [H, P, D], F32R, kind="ExternalInput").ap()
    visd = nc.dram_tensor("vis", [NSC, 128], F32, kind="ExternalInput").ap()
    onesd = nc.dram_tensor("ones_c", [128, 128], F32R, kind="ExternalInput").ap()
    outT = nc.dram_tensor("outT", [C, T0], F32, kind="ExternalOutput").ap()

    # DRAM views for slab DMAs: [part, cblock, col]
    xT_v = xT.rearrange("(cb p) t -> p cb t", p=128)
    wq_v = wqT.rearrange("(cb p) co -> p cb co", p=128)
    wk_v = wkT.rearrange("(cb p) co -> p cb co", p=128)
    wv_v = wvT.rearrange("(cb p) co -> p cb co", p=128)
    wo_v = woT.rearrange("(cb p) co -> p cb co", p=128)
    outT_v = outT.rearrange("(cb p) t -> p cb t", p=128)

    with tile.TileContext(nc) as tc, ExitStack() as ctx:
        # ---- persistent pools ----
        ktp = ctx.enter_context(tc.tile_pool(name="ktp", bufs=4))
        vp = ctx.enter_context(tc.tile_pool(name="vp", bufs=NSC))
        qtp = ctx.enter_context(tc.tile_pool(name="qtp", bufs=4))
        otp = ctx.enter_context(tc.tile_pool(name="otp", bufs=8))
        smal = ctx.enter_context(tc.tile_pool(name="smal", bufs=1))
        aux = ctx.enter_context(tc.tile_pool(name="aux", bufs=3))
        psA = ctx.enter_context(tc.tile_pool(name="psA", bufs=4, space="PSUM"))
        psB = ctx.enter_context(tc.tile_pool(name="psB", bufs=4, space="PSUM"))

        # ---- constants ----
        vis = smal.tile([128, NSC], F32, name="vis", tag="vis")
        nc.sync.dma_start(vis[:], visd.rearrange("c p -> p c"))
        bkc = smal.tile([128, NCB], F32, name="bkc", tag="bkc")
        nc.sync.dma_start(bkc[:], bkd.rearrange("(m p) -> p m", p=128))
        bqc = smal.tile([128, NCB], F32, name="bqc", tag="bqc")
        nc.sync.dma_start(bqc[:], bqd.rearrange("(m p) -> p m", p=128))
        boc = smal.tile([128, NCB], F32, name="boc", tag="boc")
        nc.sync.dma_start(boc[:], bod.rearrange("(m p) -> p m", p=128))
        bvr = smal.tile([1, C], F32R, name="bvr", tag="bvr")
        nc.sync.dma_start(bvr[:], bvd.rearrange("(a c) -> a c", a=1))
        ones_r = smal.tile([1, 128], F32R, name="ones_r", tag="ones_r")
        nc.sync.dma_start(ones_r[:], onesd[0:1, :])
        ones_b = smal.tile([65, 64], F32R, name="ones_b", tag="ones_b")
        nc.sync.dma_start(ones_b[:], onesd[0:65, 0:64])

        oT = [otp.tile([128, T0], F32R, name=f"oT{k}", tag="ot") for k in range(8)]

        for g in range(2):  # head-group pass: heads 8g..8g+8, c_out g*512..
            co0 = g * 512
            kt = [ktp.tile([128, S], F32R, name=f"kt{g}{m}", tag="kt") for m in range(4)]
            vh = [
                vp.tile([128, 8 * 65], F32R, name=f"vh{g}{c}", tag="vh") for c in range(NSC)
            ]
            qt = [qtp.tile([128, T0], F32R, name=f"qt{g}{m}", tag="qt") for m in range(4)]

            # prefix K/V + ones cols
            for lh in range(8):
                h = 8 * g + lh
                par = lh % 2
                nc.sync.dma_start(
                    kt[lh // 2][par * 64 : par * 64 + 64, 0:P], pkT[h]
                )
                nc.sync.dma_start(vh[0][0:P, lh * 65 : lh * 65 + 64], pv[h])
            for c in range(NSC):
                vv = vh[c].rearrange("p (h x) -> p h x", x=65)
                nc.sync.dma_start(
                    vv[:, :, 64:65],
                    onesd.rearrange("p (a b) -> p a b", b=1)[:, 0:8, :],
                )

            # ---- projections (streamed x and W slabs) ----
            with tc.tile_pool(name="xw", bufs=1) as xwp:
                wks = xwp.tile([128, NCB * 512], F32R, name="wks", tag="w", bufs=3)
                nc.sync.dma_start(
                    wks.rearrange("p (cb co) -> p cb co", co=512),
                    wk_v[:, :, co0 : co0 + 512],
                )
                wvs = xwp.tile([128, NCB * 512], F32R, name="wvs", tag="w", bufs=3)
                nc.sync.dma_start(
                    wvs.rearrange("p (cb co) -> p cb co", co=512),
                    wv_v[:, :, co0 : co0 + 512],
                )
                wqs = xwp.tile([128, NCB * 512], F32R, name="wqs", tag="w", bufs=3)
                nc.sync.dma_start(
                    wqs.rearrange("p (cb co) -> p cb co", co=512),
                    wq_v[:, :, co0 : co0 + 512],
                )
                wks_v = wks.rearrange("p (cb co) -> p cb co", co=512)
                wvs_v = wvs.rearrange("p (cb co) -> p cb co", co=512)
                wqs_v = wqs.rearrange("p (cb co) -> p cb co", co=512)

                for j in range(8):  # t-slabs of 256
                    xsl = xwp.tile([128, NCB * 256], F32R, name="xsl", tag="x", bufs=2)
                    nc.sync.dma_start(
                        xsl.rearrange("p (cb t) -> p cb t", t=256),
                        xT_v[:, :, j * 256 : (j + 1) * 256],
                    )
                    xv = xsl.rearrange("p (cb t) -> p cb t", t=256)

                    # K: out[c_out 128, t 256]
                    for m in range(4):
                        pk = psB.tile([128, 256], F32, name="pk", tag="psB")
                        for cb in range(NCB):
                            nc.tensor.matmul(
                                pk[:],
                                r(wks_v[:, cb, m * 128 : (m + 1) * 128]),
                                r(xv[:, cb, :]),
                                start=(cb == 0),
                                stop=(cb == NCB - 1),
                            )
                        nc.vector.tensor_scalar_add(
                            kt[m][:, P + j * 256 : P + (j + 1) * 256],
                            pk[:],
                            bkc[:, g * 4 + m : g * 4 + m + 1],
                        )

                    # V: out[t 128, c_out 512] (+bias via rank-1)
                    for tb in range(2):
                        pvp = psA.tile([128, 512], F32, name="pvp", tag="psA")
                        for cb in range(NCB):
                            nc.tensor.matmul(
                                pvp[:, 0:512],
                                r(xv[:, cb, tb * 128 : (tb + 1) * 128]),
                                r(wvs_v[:, cb, :]),
                                start=(cb == 0),
                                stop=False,
                            )
                        nc.tensor.matmul(
                            pvp[:, 0:512],
                            r(ones_r[:, 0:128]),
                            r(bvr[:, co0 : co0 + 512]),
                            start=False,
                            stop=True,
                        )
                        c = 2 * j + tb + 1
                        nc.vector.tensor_copy(
                            vh[c]
                            .rearrange("p (h x) -> p h x", x=65)[:, :, 0:64],
                            pvp[:, 0:512].rearrange("p (h x) -> p h x", x=64),
                        )

                    # Q (first half of t only): out[c_out 128, t 256]
                    if j < 4:
                        for m in range(4):
                            pq = psB.tile([128, 256], F32, name="pq", tag="psB")
                            for cb in range(NCB):
                                nc.tensor.matmul(
                                    pq[:],
                                    r(wqs_v[:, cb, m * 128 : (m + 1) * 128]),
                                    r(xv[:, cb, :]),
                                    start=(cb == 0),
                                    stop=(cb == NCB - 1),
                                )
                            nc.vector.tensor_scalar_add(
                                qt[m][:, j * 256 : (j + 1) * 256],
                                pq[:],
                                bqc[:, g * 4 + m : g * 4 + m + 1],
                            )

            # visibility mask -> V rows (and ones cols; vis^2 == vis).
            # chunk 0: only rows 0:P are ever written/read
            nc.vector.tensor_scalar_mul(vh[0][0:P, :], vh[0][0:P, :], vis[0:P, 0:1])
            for c in range(1, NSC):
                nc.vector.tensor_scalar_mul(vh[c][:], vh[c][:], vis[:, c : c + 1])

            # ---- attention ----
            # head PAIRS interleaved: even head on PE row-group 0, odd on
            # row-group 1 -> concurrent scores matmuls, denser PE stream
            # (keeps the HAM clock gate open)
            with tc.tile_pool(name="wtp", bufs=10) as wtp:
                for pr in range(4):
                    ktt = kt[pr]
                    qtt = qt[pr]
                    for tci in range(NTC):
                        t0, t1 = tci * TC, (tci + 1) * TC
                        po2 = [
                            psB.tile([128, 256], F32, name=f"po{i}", tag="psB")
                            for i in range(2)
                        ]
                        for c0, c1 in GROUPS:
                            nch = c1 - c0
                            pp = P if c0 == 0 else 128  # partitions covered
                            ps2 = [
                                psA.tile([128, 512], F32, name=f"ps{i}", tag="psA")
                                for i in range(2)
                            ]
                            for c in range(c0, c1):
                                sl = c - c0
                                if c == 0:
                                    m0, msz = 0, P
                                else:
                                    m0, msz = P + (c - 1) * 128, 128
                                for i in range(2):
                                    nc.tensor.matmul(
                                        ps2[i][0:msz, sl * TC : sl * TC + TC],
                                        r(ktt[i * 64 : i * 64 + 64, m0 : m0 + msz]),
                                        r(qtt[i * 64 : i * 64 + 64, t0:t1]),
                                        start=True,
                                        stop=True,
                                    )
                            wt2 = []
                            for i in range(2):
                                wt = wtp.tile([128, 512], F32R, name=f"wt{i}", tag="wt")
                                wt2.append(wt)
                                nc.scalar.activation(
                                    wt[0:pp, 0 : nch * TC],
                                    ps2[i][0:pp, 0 : nch * TC],
                                    mybir.ActivationFunctionType.Exp,
                                    bias=0.0,
                                    scale=0.125,
                                )
                            for c in range(c0, c1):
                                sl = c - c0
                                ksz = P if c == 0 else 128
                                for i in range(2):
                                    lh = 2 * pr + i
                                    nc.tensor.matmul(
                                        po2[i][0:65, :],
                                        r(vh[c][0:ksz, lh * 65 : lh * 65 + 65]),
                                        r(wt2[i][0:ksz, sl * TC : sl * TC + TC]),
                                        start=(c == 0),
                                        stop=(c == NSC - 1),
                                    )
                        for i in range(2):
                            po = po2[i]
                            gh = 8 * g + 2 * pr + i
                            # normalize: row 64 of po = denominator
                            rcp = aux.tile([65, 256], F32R, name="rcp", tag="rcp")
                            with nc.allow_low_precision(reason="f32r == f32 bits"):
                                nc.vector.reciprocal(rcp[64:65, :], po[64:65, :])
                            pb = psA.tile([128, 512], F32, name="pb", tag="psA")
                            nc.tensor.matmul(
                                pb[0:64, 0:256],
                                r(ones_b[64:65, :]),
                                r(rcp[64:65, :]),
                                start=True,
                                stop=True,
                            )
                            rb = aux.tile([64, 256], F32, name="rb", tag="rb")
                            nc.vector.tensor_copy(rb[:], pb[0:64, 0:256])
                            ot = oT[gh // 2]
                            if gh % 2 == 0:
                                nc.vector.tensor_mul(
                                    ot[0:64, t0:t1], po[0:64, :], rb[:]
                                )
                            else:
                                tmp = aux.tile([64, 256], F32R, name="tmp", tag="tmp")
                                nc.vector.tensor_mul(tmp[:], po[0:64, :], rb[:])
                                nc.sync.dma_start(ot[64:128, t0:t1], tmp[:])

        # ---- output projection: outT[c_out, t] = WoT.T @ oT (+bo) ----
        with tc.tile_pool(name="wop", bufs=2) as wop:
            for hf in range(2):
                wos = wop.tile([128, NCB * 512], F32R, name="wos", tag="wo")
                nc.sync.dma_start(
                    wos.rearrange("p (cb co) -> p cb co", co=512),
                    wo_v[:, :, hf * 512 : (hf + 1) * 512],
                )
                wos_v = wos.rearrange("p (cb co) -> p cb co", co=512)
                for mo in range(4):
                    cbo = hf * 4 + mo
                    for tj in range(2):
                        pO = psA.tile([128, 512], F32, name="pO", tag="psA")
                        for cb in range(NCB):
                            nc.tensor.matmul(
                                pO[:, 0:512],
                                r(wos_v[:, cb, mo * 128 : (mo + 1) * 128]),
                                r(oT[cb][:, tj * 512 : (tj + 1) * 512]),
                                start=(cb == 0),
                                stop=(cb == NCB - 1),
                            )
                        st = qtp.tile([128, T0], F32, name="st", tag="qt")
                        nc.vector.tensor_scalar_add(
                            st[:, 0:512], pO[:, 0:512], boc[:, cbo : cbo + 1]
                        )
                        nc.sync.dma_start(
                            outT_v[:, cbo, tj * 512 : (tj + 1) * 512],
                            st[:, 0:512],
                        )
    nc.compile()
    return nc


_NC = None


def _get_nc():
    global _NC
    if _NC is None:
        _NC = build_nc()
    return _NC


def make_in_maps(x, attn_mask, prefix_k, prefix_v, Wq, bq, Wk, bk, Wv, bv, Wo, bo):
    shared = {
        "wqT": np.ascontiguousarray(np.asarray(Wq, np.float32).T),
        "wkT": np.ascontiguousarray(np.asarray(Wk, np.float32).T),
        "wvT": np.ascontiguousarray(np.asarray(Wv, np.float32).T),
        "woT": np.ascontiguousarray(np.asarray(Wo, np.float32).T),
        "bq": np.asarray(bq, np.float32),
        "bk": np.asarray(bk, np.float32),
        "bv": np.asarray(bv, np.float32),
        "bo": np.asarray(bo, np.float32),
    }
    x = np.asarray(x, np.float32)
    attn_mask = np.asarray(attn_mask)
    prefix_k = np.asarray(prefix_k, np.float32)
    prefix_v = np.asarray(prefix_v, np.float32)
    ones_c = np.ones((128, 128), np.float32)
    in_maps = []
    for core in range(8):
        b, half = core // 2, core % 2
        # query half first, other half second (keeps the device program SPMD)
        xp = np.concatenate(
            [x[b, half * T0 : (half + 1) * T0], x[b, (1 - half) * T0 : (2 - half) * T0]]
        )
        m = attn_mask[b, 0, 0].astype(np.float32)
        mperm = np.concatenate(
            [m[half * T0 : (half + 1) * T0], m[(1 - half) * T0 : (2 - half) * T0]]
        )
        vis = np.zeros((NSC, 128), np.float32)
        vis[0, :P] = 1.0
        vis[1:] = mperm.reshape(16, 128)
        in_maps.append(
            dict(
                shared,
                xT=np.ascontiguousarray(xp.T),
                pkT=np.ascontiguousarray(prefix_k[b].transpose(0, 2, 1)),
                pv=np.ascontiguousarray(prefix_v[b]),
                vis=vis,
                ones_c=ones_c,
            )
        )
    return in_maps


def kernel(**inputs) -> np.ndarray:
    nc = _get_nc()
    in_maps = make_in_maps(**inputs)
    res = bass_utils.run_bass_kernel_spmd(nc, in_maps, core_ids=list(range(8)))
    out = np.empty((B, T, C), np.float32)
    for core in range(8):
        b, half = core // 2, core % 2
        out[b, half * T0 : (half + 1) * T0] = res.results[core]["outT"].T
    return out
